# revision 1
# baseline (speedup 1.0000x reference)
"""Trainium2 Bass kernel for the VAE-style loss function.

Computes, from full inputs
    x, x_out: [256, 3, 128, 128] f32
    y:        [256, 7]  f32 (integer labels 0..9 with NaN = unlabeled)
    mu:       [256, 32] f32
    disc_pos: [10]      f32
the three scalars (recon, kld, recon + kld) exactly as the reference:
    recon   = |x - x_out|.sum(axis=(1,2,3)).mean()
    kld_d   = where(isnan(y_d), min_p (mu_d - pos_p)^2, (mu_d - pos[y_d])^2).mean(0).sum()
    kld_l   = where(isnan(y_l), relu(|mu_l| - 10)^2, (mu_l - y_l)^2).sum(1).mean()
    kld     = kld_d + kld_l

Strategy: pure data parallel over the batch dim across 8 NeuronCores.
Each core reduces its 32-sample slice to three partial sums (one SPMD
program, per-core input slices), and the host sums the 8 x 3 partials
and divides by 256.
"""

import numpy as np

import concourse.bass as bass
import concourse.mybir as mybir
import concourse.bacc as bacc
import concourse.tile as tile


F32 = mybir.dt.float32
ALU = mybir.AluOpType
AXIS = mybir.AxisListType

N_CORES = 8
B = 256
BL = B // N_CORES          # 32 samples per core
P = 128                    # SBUF partitions
TOT = BL * 3 * 128 * 128   # 1572864 elements per big tensor per core
FREE = TOT // P            # 12288 elements per partition
NCHUNK = 6
CH = FREE // NCHUNK        # 2048
ND = 3                     # discrete dims
NL = 4                     # linear dims
NPOS = 10                  # codebook positions


# smalls packing: [BL, 32 mu | 7 y | 10 disc_pos bcast | 10 iota] = [32, 59]
SM_MU = 0
SM_Y = 32
SM_POS = 39
SM_IOTA = 49
SM_W = 59


def build_module():
    nc = bacc.Bacc(
        "TRN2", target_bir_lowering=False, debug=False, num_devices=N_CORES
    )
    # x and x_out stacked host-side so each chunk is a single DMA (the
    # DVE TensorTensor ISA slot only fits one sync wait).
    xc = nc.dram_tensor("xc", [2, TOT], F32, kind="ExternalInput")
    sm = nc.dram_tensor("smalls", [BL, SM_W], F32, kind="ExternalInput")
    out = nc.dram_tensor("out", [1, 3], F32, kind="ExternalOutput")

    # [2, TOT] -> [p, 2, n]: partition-major within each half
    xcf = xc.ap().rearrange("h (p n) -> p h n", p=P)

    with tile.TileContext(nc) as tc:
        with (
            tc.tile_pool(name="big", bufs=NCHUNK) as bp,
            tc.tile_pool(name="acc", bufs=1) as cp,
            tc.tile_pool(name="small", bufs=1) as sp,
            tc.tile_pool(name="work", bufs=2) as wp,
            tc.tile_pool(name="psum", bufs=1, space="PSUM") as pp,
        ):
            # ---------------- recon: sum |x - x_out| ----------------
            acc = cp.tile([P, NCHUNK], F32)
            for i in range(NCHUNK):
                xt = bp.tile([P, 2, CH], F32, tag="xt")
                nc.sync.dma_start(out=xt[:], in_=xcf[:, :, i * CH : (i + 1) * CH])
                nc.vector.tensor_sub(xt[:, 0, :], xt[:, 0, :], xt[:, 1, :])
                nc.vector.tensor_reduce(
                    acc[:, i : i + 1],
                    xt[:, 0, :],
                    AXIS.X,
                    ALU.add,
                    apply_absolute_value=True,
                )
            # stk collects per-partition partials: col0 recon (128 rows),
            # col1 disc, col2 lin (32 rows each, rest zero)
            stk = cp.tile([P, 3], F32)
            nc.vector.memset(stk[:], 0.0)
            nc.vector.tensor_reduce(stk[:, 0:1], acc[:], AXIS.X, ALU.add)

            # ---------------- small tensors (one DMA) ----------------
            sm_t = sp.tile([BL, SM_W], F32)
            nc.sync.dma_start(out=sm_t[:], in_=sm.ap())
            mu_t = sm_t[:, SM_MU : SM_MU + 32]
            y_t = sm_t[:, SM_Y : SM_Y + ND + NL]
            posb = sm_t[:, SM_POS : SM_POS + NPOS]
            iot = sm_t[:, SM_IOTA : SM_IOTA + NPOS]

            accd = sp.tile([BL, 1], F32)
            nc.vector.memset(accd[:], 0.0)
            accl = sp.tile([BL, 1], F32)
            nc.vector.memset(accl[:], 0.0)

            # ---------------- discrete KLD ----------------
            # per dim d: sel = isnan(y) ? min_p (mu-pos_p)^2 : (mu-pos[y])^2
            for d in range(ND):
                mu_c = mu_t[:, d : d + 1]
                y_c = y_t[:, d : d + 1]
                diff = wp.tile([BL, NPOS], F32, tag="w10a")
                nc.vector.tensor_scalar(diff[:], posb, mu_c, None, ALU.subtract)
                dist = wp.tile([BL, NPOS], F32, tag="w10b")
                nc.vector.tensor_mul(dist[:], diff[:], diff[:])
                unl = wp.tile([BL, 1], F32, tag="w1a")
                nc.vector.tensor_reduce(unl[:], dist[:], AXIS.X, ALU.min)
                oh = wp.tile([BL, NPOS], F32, tag="w10c")
                nc.vector.tensor_scalar(oh[:], iot, y_c, None, ALU.is_equal)
                labt = wp.tile([BL, NPOS], F32, tag="w10d")
                nc.vector.tensor_mul(labt[:], dist[:], oh[:])
                lab = wp.tile([BL, 1], F32, tag="w1b")
                nc.vector.tensor_reduce(lab[:], labt[:], AXIS.X, ALU.add)
                eq = wp.tile([BL, 1], F32, tag="w1c")
                nc.vector.tensor_scalar(eq[:], y_c, y_c, None, ALU.is_equal)
                # sel = unl + (lab - unl) * eq
                t1 = wp.tile([BL, 1], F32, tag="w1d")
                nc.vector.tensor_sub(t1[:], lab[:], unl[:])
                t2 = wp.tile([BL, 1], F32, tag="w1e")
                nc.vector.tensor_mul(t2[:], t1[:], eq[:])
                nc.vector.tensor_add(t2[:], t2[:], unl[:])
                nc.vector.tensor_add(accd[:], accd[:], t2[:])

            # ---------------- linear KLD ----------------
            # per dim l: sel = isnan(y) ? relu(|mu|-10)^2 : (mu-y)^2
            for l in range(NL):
                c = ND + l
                mu_c = mu_t[:, c : c + 1]
                y_c = y_t[:, c : c + 1]
                # y_safe = sum_p p * (y == p)  (0 when y is NaN; exact for int y)
                oh = wp.tile([BL, NPOS], F32, tag="w10a")
                nc.vector.tensor_scalar(oh[:], iot, y_c, None, ALU.is_equal)
                yst = wp.tile([BL, NPOS], F32, tag="w10b")
                nc.vector.tensor_mul(yst[:], oh[:], iot)
                ysafe = wp.tile([BL, 1], F32, tag="w1a")
                nc.vector.tensor_reduce(ysafe[:], yst[:], AXIS.X, ALU.add)
                d1 = wp.tile([BL, 1], F32, tag="w1b")
                nc.vector.tensor_sub(d1[:], mu_c, ysafe[:])
                lab = wp.tile([BL, 1], F32, tag="w1c")
                nc.vector.tensor_mul(lab[:], d1[:], d1[:])
                # nolabel = relu(|mu| - 10)^2, |mu| = max(mu, -mu)
                nm = wp.tile([BL, 1], F32, tag="w1j")
                nc.vector.tensor_scalar(nm[:], mu_c, -1.0, None, ALU.mult)
                a = wp.tile([BL, 1], F32, tag="w1d")
                nc.vector.tensor_max(a[:], mu_c, nm[:])
                r = wp.tile([BL, 1], F32, tag="w1e")
                nc.vector.tensor_scalar(r[:], a[:], -10.0, 0.0, ALU.add, ALU.max)
                n = wp.tile([BL, 1], F32, tag="w1f")
                nc.vector.tensor_mul(n[:], r[:], r[:])
                eq = wp.tile([BL, 1], F32, tag="w1g")
                nc.vector.tensor_scalar(eq[:], y_c, y_c, None, ALU.is_equal)
                # sel = n + (lab - n) * eq
                t1 = wp.tile([BL, 1], F32, tag="w1h")
                nc.vector.tensor_sub(t1[:], lab[:], n[:])
                t2 = wp.tile([BL, 1], F32, tag="w1i")
                nc.vector.tensor_mul(t2[:], t1[:], eq[:])
                nc.vector.tensor_add(t2[:], t2[:], n[:])
                nc.vector.tensor_add(accl[:], accl[:], t2[:])

            # ---------------- partial-sum outputs ----------------
            # partition-reduce all three columns at once: ones.T @ stk -> [1,3]
            nc.vector.tensor_copy(stk[0:BL, 1:2], accd[:])
            nc.vector.tensor_copy(stk[0:BL, 2:3], accl[:])
            ones_t = sp.tile([P, 1], F32)
            nc.vector.memset(ones_t[:], 1.0)
            ps = pp.tile([1, 3], F32)
            nc.tensor.matmul(ps[:], ones_t[:], stk[:], start=True, stop=True)
            res = sp.tile([1, 3], F32)
            nc.vector.tensor_copy(res[:], ps[:])
            nc.sync.dma_start(out=out.ap(), in_=res[:])

    nc.compile()
    return nc


_NC_CACHE = None


def _get_module():
    global _NC_CACHE
    if _NC_CACHE is None:
        _NC_CACHE = build_module()
    return _NC_CACHE


def make_in_maps(x, x_out, y, mu, disc_pos):
    x = np.ascontiguousarray(x, dtype=np.float32)
    x_out = np.ascontiguousarray(x_out, dtype=np.float32)
    y = np.ascontiguousarray(y, dtype=np.float32)
    mu = np.ascontiguousarray(mu, dtype=np.float32)
    disc_pos = np.ascontiguousarray(disc_pos, dtype=np.float32)
    posb = np.tile(disc_pos.reshape(1, NPOS), (BL, 1))
    iota = np.tile(np.arange(NPOS, dtype=np.float32), (BL, 1))
    in_maps = []
    for i in range(N_CORES):
        s = slice(i * BL, (i + 1) * BL)
        xcore = np.empty((2, TOT), dtype=np.float32)
        xcore[0] = x[s].reshape(-1)
        xcore[1] = x_out[s].reshape(-1)
        smalls = np.concatenate([mu[s], y[s], posb, iota], axis=1).astype(
            np.float32
        )
        assert smalls.shape == (BL, SM_W)
        in_maps.append({"xc": xcore, "smalls": smalls})
    return in_maps


def combine_partials(partials):
    """partials: [8, 1, 3] (or [8, 3]) per-core sums -> full (3,) output."""
    p = np.asarray(partials, dtype=np.float64).reshape(N_CORES, 3)
    s = p.sum(axis=0) / B
    recon = s[0]
    kld = s[1] + s[2]
    return np.array([recon, kld, recon + kld], dtype=np.float32)


def run_spmd(x, x_out, y, mu, disc_pos, trace=False, **kw):
    from concourse.bass_utils import run_bass_kernel_spmd

    nc = _get_module()
    in_maps = make_in_maps(x, x_out, y, mu, disc_pos)
    r = run_bass_kernel_spmd(nc, in_maps, list(range(N_CORES)), trace=trace, **kw)
    partials = [r.results[i]["out"] for i in range(N_CORES)]
    return combine_partials(partials), r


def kernel(x, x_out, y, mu, disc_pos):
    out, _ = run_spmd(x, x_out, y, mu, disc_pos)
    return out


if __name__ == "__main__":
    nc = build_module()
    print("module built ok")



# revision 2
# speedup vs baseline: 1.1687x; 1.1687x over previous
"""Trainium2 Bass kernel for the VAE-style loss function.

Computes, from full inputs
    x, x_out: [256, 3, 128, 128] f32
    y:        [256, 7]  f32 (integer labels 0..9 with NaN = unlabeled)
    mu:       [256, 32] f32
    disc_pos: [10]      f32
the three scalars (recon, kld, recon + kld) exactly as the reference:
    recon   = |x - x_out|.sum(axis=(1,2,3)).mean()
    kld_d   = where(isnan(y_d), min_p (mu_d - pos_p)^2, (mu_d - pos[y_d])^2).mean(0).sum()
    kld_l   = where(isnan(y_l), relu(|mu_l| - 10)^2, (mu_l - y_l)^2).sum(1).mean()
    kld     = kld_d + kld_l

Strategy: pure data parallel over the batch dim across 8 NeuronCores.
Each core reduces its 32-sample slice to per-partition partial sums
(one SPMD program, per-core input slices); the host sums the partials.

Per-core schedule (v2):
  - smalls (mu/y/codebook helpers) DMA'd first on the Scalar-engine
    HWDGE queue so it never queues behind the 12.6 MB bulk stream.
  - KLD computed as ~22 batched wide DVE ops over [32, <=40] tiles,
    issued before the chunk loop so it hides in the DMA ramp-up.
  - recon pipelined over 12 chunks: Sync-queue DMA -> DVE in-place
    subtract -> Scalar-engine fused Abs+accumulate into a column of
    the partial-sum tile. Splitting sub (DVE) and abs+sum (Act)
    halves the per-chunk Vector load and shortens the tail.
  - one wide DMA of the [128, 14] partial tile out; host finishes.
"""

import numpy as np

import concourse.bass as bass
import concourse.mybir as mybir
import concourse.bacc as bacc
import concourse.tile as tile

F32 = mybir.dt.float32
ALU = mybir.AluOpType
AXIS = mybir.AxisListType
AF = mybir.ActivationFunctionType

N_CORES = 8
B = 256
BL = B // N_CORES          # 32 samples per core
P = 128                    # SBUF partitions
FREE = BL * 3 * 128 * 128 // P   # 12288 elements per partition per tensor
NCHUNK = 12
CH = FREE // NCHUNK        # 1024
ND = 3                     # discrete dims
NL = 4                     # linear dims
NPOS = 10                  # codebook positions
NCOL = NCHUNK + 2          # partial-sum columns: 12 recon + disc + lin

# smalls packing: [BL, 211]
MU_R = 0                   # mu_d repeated x10      (30)
POS3 = 30                  # disc_pos tiled x3      (30)
Y_R = 60                   # y_d repeated x10       (30)
IOTA3 = 90                 # iota(10) tiled x3      (30)
YL_R = 120                 # y_l repeated x10       (40)
IOTA4 = 160                # iota(10) tiled x4      (40)
MU_L = 200                 # mu_l                   (4)
Y_L = 204                  # y_l                    (4)
Y_D = 208                  # y_d                    (3)
SM_W = 211


def build_module():
    nc = bacc.Bacc(
        "TRN2", target_bir_lowering=False, debug=False, num_devices=N_CORES
    )
    # Chunk-contiguous layout: chunk i is xc[i], one 8 KB descriptor per
    # SBUF partition ([2, CH] = 8 KB contiguous both sides).
    xc = nc.dram_tensor("xc", [NCHUNK, P, 2 * CH], F32, kind="ExternalInput")
    sm = nc.dram_tensor("smalls", [BL, SM_W], F32, kind="ExternalInput")
    out = nc.dram_tensor("out", [P, NCOL], F32, kind="ExternalOutput")

    xcf = xc.ap()

    with tile.TileContext(nc) as tc:
        with (
            tc.tile_pool(name="big", bufs=NCHUNK) as bp,
            tc.tile_pool(name="small", bufs=1) as sp,
            tc.tile_pool(name="work", bufs=1) as wp,
        ):
            # smalls first, on the Act HWDGE queue (bulk uses Sync's)
            sm_t = sp.tile([BL, SM_W], F32)
            nc.scalar.dma_start(out=sm_t[:], in_=sm.ap())

            stk = sp.tile([P, NCOL], F32)
            nc.vector.memset(stk[:], 0.0)

            # ---------------- discrete KLD (batched over dims) -------
            # dist[b, d, p] = (mu_d[b,d] - pos[p])^2 laid out [32, 30]
            dd = wp.tile([BL, ND * NPOS], F32, tag="dd")
            nc.vector.tensor_sub(
                dd[:], sm_t[:, MU_R : MU_R + 30], sm_t[:, POS3 : POS3 + 30]
            )
            dist = wp.tile([BL, ND * NPOS], F32, tag="dist")
            nc.vector.tensor_mul(dist[:], dd[:], dd[:])
            unl = wp.tile([BL, ND], F32, tag="unl")
            nc.vector.tensor_reduce(
                unl[:],
                dist[:].rearrange("p (d q) -> p d q", q=NPOS),
                AXIS.X,
                ALU.min,
            )
            # labeled: one-hot(y) picks dist at the assigned position
            oh = wp.tile([BL, ND * NPOS], F32, tag="oh")
            nc.vector.tensor_tensor(
                oh[:], sm_t[:, IOTA3 : IOTA3 + 30], sm_t[:, Y_R : Y_R + 30],
                ALU.is_equal,
            )
            nc.vector.tensor_mul(dd[:], dist[:], oh[:])
            lab = wp.tile([BL, ND], F32, tag="lab")
            nc.vector.tensor_reduce(
                lab[:],
                dd[:].rearrange("p (d q) -> p d q", q=NPOS),
                AXIS.X,
                ALU.add,
            )
            # eq = 1 where labeled (y==y fails for NaN)
            eq = wp.tile([BL, ND], F32, tag="eq")
            nc.vector.tensor_tensor(
                eq[:], sm_t[:, Y_D : Y_D + ND], sm_t[:, Y_D : Y_D + ND],
                ALU.is_equal,
            )
            # sel = unl + (lab - unl) * eq ; accd = sum over dims
            nc.vector.tensor_sub(lab[:], lab[:], unl[:])
            nc.vector.tensor_mul(lab[:], lab[:], eq[:])
            nc.vector.tensor_add(lab[:], lab[:], unl[:])
            nc.vector.tensor_reduce(
                stk[0:BL, NCHUNK : NCHUNK + 1], lab[:], AXIS.X, ALU.add
            )

            # ---------------- linear KLD (batched over dims) ---------
            # y_safe = sum_p p * (y == p)  (0 when y is NaN; exact for int y)
            ohl = wp.tile([BL, NL * NPOS], F32, tag="ohl")
            nc.vector.tensor_tensor(
                ohl[:], sm_t[:, IOTA4 : IOTA4 + 40], sm_t[:, YL_R : YL_R + 40],
                ALU.is_equal,
            )
            nc.vector.tensor_mul(ohl[:], ohl[:], sm_t[:, IOTA4 : IOTA4 + 40])
            ysafe = wp.tile([BL, NL], F32, tag="ysafe")
            nc.vector.tensor_reduce(
                ysafe[:],
                ohl[:].rearrange("p (d q) -> p d q", q=NPOS),
                AXIS.X,
                ALU.add,
            )
            nc.vector.tensor_sub(ysafe[:], sm_t[:, MU_L : MU_L + NL], ysafe[:])
            lab2 = wp.tile([BL, NL], F32, tag="lab2")
            nc.vector.tensor_mul(lab2[:], ysafe[:], ysafe[:])
            # nolabel = relu(|mu| - 10)^2, |mu| = max(mu, -mu)
            nm = wp.tile([BL, NL], F32, tag="nm")
            nc.vector.tensor_scalar(
                nm[:], sm_t[:, MU_L : MU_L + NL], -1.0, None, ALU.mult
            )
            nc.vector.tensor_max(nm[:], sm_t[:, MU_L : MU_L + NL], nm[:])
            nc.vector.tensor_scalar(nm[:], nm[:], -10.0, 0.0, ALU.add, ALU.max)
            nc.vector.tensor_mul(nm[:], nm[:], nm[:])
            eq2 = wp.tile([BL, NL], F32, tag="eq2")
            nc.vector.tensor_tensor(
                eq2[:], sm_t[:, Y_L : Y_L + NL], sm_t[:, Y_L : Y_L + NL],
                ALU.is_equal,
            )
            nc.vector.tensor_sub(lab2[:], lab2[:], nm[:])
            nc.vector.tensor_mul(lab2[:], lab2[:], eq2[:])
            nc.vector.tensor_add(lab2[:], lab2[:], nm[:])
            nc.vector.tensor_reduce(
                stk[0:BL, NCHUNK + 1 : NCHUNK + 2], lab2[:], AXIS.X, ALU.add
            )

            # ---------------- recon: sum |x - x_out| -----------------
            for i in range(NCHUNK):
                xt = bp.tile([P, 2, CH], F32, tag="xt")
                nc.sync.dma_start(out=xt[:], in_=xcf[i])
                nc.vector.tensor_sub(xt[:, 0, :], xt[:, 0, :], xt[:, 1, :])
                nc.scalar.activation(
                    xt[:, 0, :],
                    xt[:, 0, :],
                    AF.Abs,
                    accum_out=stk[:, i : i + 1],
                )

            nc.scalar.dma_start(out=out.ap(), in_=stk[:])

    nc.compile()
    return nc


_NC_CACHE = None


def _get_module():
    global _NC_CACHE
    if _NC_CACHE is None:
        _NC_CACHE = build_module()
    return _NC_CACHE


def make_in_maps(x, x_out, y, mu, disc_pos):
    x = np.ascontiguousarray(x, dtype=np.float32)
    x_out = np.ascontiguousarray(x_out, dtype=np.float32)
    y = np.asarray(y, dtype=np.float32)
    mu = np.asarray(mu, dtype=np.float32)
    disc_pos = np.asarray(disc_pos, dtype=np.float32)

    iota = np.arange(NPOS, dtype=np.float32)
    pos3 = np.tile(np.tile(disc_pos, ND)[None, :], (BL, 1))
    iota3 = np.tile(np.tile(iota, ND)[None, :], (BL, 1))
    iota4 = np.tile(np.tile(iota, NL)[None, :], (BL, 1))

    in_maps = []
    for i in range(N_CORES):
        s = slice(i * BL, (i + 1) * BL)
        xcore = np.empty((NCHUNK, P, 2, CH), dtype=np.float32)
        xcore[:, :, 0, :] = x[s].reshape(P, NCHUNK, CH).transpose(1, 0, 2)
        xcore[:, :, 1, :] = x_out[s].reshape(P, NCHUNK, CH).transpose(1, 0, 2)

        mu_d, mu_l = mu[s, :ND], mu[s, ND : ND + NL]
        y_d, y_l = y[s, :ND], y[s, ND : ND + NL]
        smalls = np.concatenate(
            [
                np.repeat(mu_d, NPOS, axis=1),   # MU_R
                pos3,                            # POS3
                np.repeat(y_d, NPOS, axis=1),    # Y_R
                iota3,                           # IOTA3
                np.repeat(y_l, NPOS, axis=1),    # YL_R
                iota4,                           # IOTA4
                mu_l,                            # MU_L
                y_l,                             # Y_L
                y_d,                             # Y_D
            ],
            axis=1,
        ).astype(np.float32)
        assert smalls.shape == (BL, SM_W)
        in_maps.append({"xc": xcore.reshape(NCHUNK, P, 2 * CH), "smalls": smalls})
    return in_maps


def combine_partials(partials):
    """partials: [8, P, NCOL] per-core column sums -> full (3,) output."""
    p = np.asarray(partials, dtype=np.float64).reshape(N_CORES, P, NCOL)
    s = p.sum(axis=(0, 1)) / B
    recon = s[:NCHUNK].sum()
    kld = s[NCHUNK] + s[NCHUNK + 1]
    return np.array([recon, kld, recon + kld], dtype=np.float32)


def run_spmd(x, x_out, y, mu, disc_pos, trace=False, **kw):
    from concourse.bass_utils import run_bass_kernel_spmd

    nc = _get_module()
    in_maps = make_in_maps(x, x_out, y, mu, disc_pos)
    r = run_bass_kernel_spmd(nc, in_maps, list(range(N_CORES)), trace=trace, **kw)
    partials = [r.results[i]["out"] for i in range(N_CORES)]
    return combine_partials(partials), r


def kernel(x, x_out, y, mu, disc_pos):
    out, _ = run_spmd(x, x_out, y, mu, disc_pos)
    return out


if __name__ == "__main__":
    nc = build_module()
    print("module built ok")


# revision 8
# speedup vs baseline: 1.3793x; 1.1802x over previous
"""Trainium2 Bass kernel for the VAE-style loss function.

Computes, from full inputs
    x, x_out: [256, 3, 128, 128] f32
    y:        [256, 7]  f32 (integer labels 0..9 with NaN = unlabeled)
    mu:       [256, 32] f32
    disc_pos: [10]      f32
the three scalars (recon, kld, recon + kld) exactly as the reference:
    recon   = |x - x_out|.sum(axis=(1,2,3)).mean()
    kld_d   = where(isnan(y_d), min_p (mu_d - pos_p)^2, (mu_d - pos[y_d])^2).mean(0).sum()
    kld_l   = where(isnan(y_l), relu(|mu_l| - 10)^2, (mu_l - y_l)^2).sum(1).mean()
    kld     = kld_d + kld_l

Strategy: pure data parallel over the batch dim across 8 NeuronCores.
Each core reduces its 32-sample slice to per-partition partial sums
(one SPMD program, per-core input slices); the host sums the partials.

Per-core schedule (v2):
  - smalls (mu/y/codebook helpers) DMA'd first on the Scalar-engine
    HWDGE queue so it never queues behind the 12.6 MB bulk stream.
  - KLD computed as ~22 batched wide DVE ops over [32, <=40] tiles,
    issued before the chunk loop so it hides in the DMA ramp-up.
  - recon pipelined over 12 chunks: Sync-queue DMA -> DVE in-place
    subtract -> Scalar-engine fused Abs+accumulate into a column of
    the partial-sum tile. Splitting sub (DVE) and abs+sum (Act)
    halves the per-chunk Vector load and shortens the tail.
  - one wide DMA of the [128, 14] partial tile out; host finishes.
"""

import numpy as np

import concourse.bass as bass
import concourse.mybir as mybir
import concourse.bacc as bacc
import concourse.tile as tile

F32 = mybir.dt.float32
ALU = mybir.AluOpType
AXIS = mybir.AxisListType
AF = mybir.ActivationFunctionType

N_CORES = 8
B = 256
BL = B // N_CORES          # 32 samples per core
P = 128                    # SBUF partitions
FREE = BL * 3 * 128 * 128 // P   # 12288 elements per partition per tensor
CH = 1024                  # main chunk width
NBIG = 11                  # 11 chunks of 1024
CHS = 512                  # tail chunk width
NSMALL = 2                 # 2 chunks of 512 (short post-land chain)
NCHUNK = NBIG + NSMALL
ND = 3                     # discrete dims
NL = 4                     # linear dims
NPOS = 10                  # codebook positions
NCOL = NCHUNK + 2          # partial-sum columns: 13 recon + disc + lin

# smalls packing: [BL, 211]
MU_R = 0                   # mu_d repeated x10      (30)
POS3 = 30                  # disc_pos tiled x3      (30)
Y_R = 60                   # y_d repeated x10       (30)
IOTA3 = 90                 # iota(10) tiled x3      (30)
YL_R = 120                 # y_l repeated x10       (40)
IOTA4 = 160                # iota(10) tiled x4      (40)
MU_L = 200                 # mu_l                   (4)
Y_L = 204                  # y_l                    (4)
Y_D = 208                  # y_d                    (3)
SM_W = 211


def build_module():
    nc = bacc.Bacc(
        "TRN2", target_bir_lowering=False, debug=False, num_devices=N_CORES
    )
    # Chunk-contiguous layout: chunk i is xc1[i], one 8 KB descriptor per
    # SBUF partition ([2, CH] = 8 KB contiguous both sides). xc2 holds two
    # half-width tail chunks so the last land->reduce chain is short.
    xc1 = nc.dram_tensor("xc1", [NBIG, P, 2 * CH], F32, kind="ExternalInput")
    xc2 = nc.dram_tensor("xc2", [NSMALL, P, 2 * CHS], F32, kind="ExternalInput")
    sm = nc.dram_tensor("smalls", [BL, SM_W], F32, kind="ExternalInput")
    out = nc.dram_tensor("out", [P, NCOL], F32, kind="ExternalOutput")

    xcf1 = xc1.ap()
    xcf2 = xc2.ap()

    with tile.TileContext(nc) as tc:
        with (
            tc.tile_pool(name="big", bufs=NCHUNK) as bp,
            tc.tile_pool(name="small", bufs=1) as sp,
            tc.tile_pool(name="work", bufs=1) as wp,
        ):
            # smalls first, on the GpSimd SWDGE queue: its descriptors are
            # consumed before the Sync-queue bulk stream appears, so the
            # DMA engines never ring-switch mid-bulk.
            sm_t = sp.tile([BL, SM_W], F32)
            nc.gpsimd.dma_start(out=sm_t[:], in_=sm.ap())

            stk = sp.tile([P, NCOL], F32)
            nc.vector.memset(stk[:], 0.0)

            # ---------------- discrete KLD (batched over dims) -------
            # dist[b, d, p] = (mu_d[b,d] - pos[p])^2 laid out [32, 30]
            dd = wp.tile([BL, ND * NPOS], F32, tag="dd")
            nc.vector.tensor_sub(
                dd[:], sm_t[:, MU_R : MU_R + 30], sm_t[:, POS3 : POS3 + 30]
            )
            dist = wp.tile([BL, ND * NPOS], F32, tag="dist")
            nc.vector.tensor_mul(dist[:], dd[:], dd[:])
            unl = wp.tile([BL, ND], F32, tag="unl")
            nc.vector.tensor_reduce(
                unl[:],
                dist[:].rearrange("p (d q) -> p d q", q=NPOS),
                AXIS.X,
                ALU.min,
            )
            # labeled: one-hot(y) picks dist at the assigned position
            oh = wp.tile([BL, ND * NPOS], F32, tag="oh")
            nc.vector.tensor_tensor(
                oh[:], sm_t[:, IOTA3 : IOTA3 + 30], sm_t[:, Y_R : Y_R + 30],
                ALU.is_equal,
            )
            nc.vector.tensor_mul(dd[:], dist[:], oh[:])
            lab = wp.tile([BL, ND], F32, tag="lab")
            nc.vector.tensor_reduce(
                lab[:],
                dd[:].rearrange("p (d q) -> p d q", q=NPOS),
                AXIS.X,
                ALU.add,
            )
            # eq = 1 where labeled (y==y fails for NaN)
            eq = wp.tile([BL, ND], F32, tag="eq")
            nc.vector.tensor_tensor(
                eq[:], sm_t[:, Y_D : Y_D + ND], sm_t[:, Y_D : Y_D + ND],
                ALU.is_equal,
            )
            # sel = unl + (lab - unl) * eq ; accd = sum over dims
            nc.vector.tensor_sub(lab[:], lab[:], unl[:])
            nc.vector.tensor_mul(lab[:], lab[:], eq[:])
            nc.vector.tensor_add(lab[:], lab[:], unl[:])
            nc.vector.tensor_reduce(
                stk[0:BL, NCHUNK : NCHUNK + 1], lab[:], AXIS.X, ALU.add
            )

            # ---------------- linear KLD (batched over dims) ---------
            # y_safe = sum_p p * (y == p)  (0 when y is NaN; exact for int y)
            ohl = wp.tile([BL, NL * NPOS], F32, tag="ohl")
            nc.vector.tensor_tensor(
                ohl[:], sm_t[:, IOTA4 : IOTA4 + 40], sm_t[:, YL_R : YL_R + 40],
                ALU.is_equal,
            )
            nc.vector.tensor_mul(ohl[:], ohl[:], sm_t[:, IOTA4 : IOTA4 + 40])
            ysafe = wp.tile([BL, NL], F32, tag="ysafe")
            nc.vector.tensor_reduce(
                ysafe[:],
                ohl[:].rearrange("p (d q) -> p d q", q=NPOS),
                AXIS.X,
                ALU.add,
            )
            nc.vector.tensor_sub(ysafe[:], sm_t[:, MU_L : MU_L + NL], ysafe[:])
            lab2 = wp.tile([BL, NL], F32, tag="lab2")
            nc.vector.tensor_mul(lab2[:], ysafe[:], ysafe[:])
            # nolabel = relu(|mu| - 10)^2, |mu| = max(mu, -mu)
            nm = wp.tile([BL, NL], F32, tag="nm")
            nc.vector.tensor_scalar(
                nm[:], sm_t[:, MU_L : MU_L + NL], -1.0, None, ALU.mult
            )
            nc.vector.tensor_max(nm[:], sm_t[:, MU_L : MU_L + NL], nm[:])
            nc.vector.tensor_scalar(nm[:], nm[:], -10.0, 0.0, ALU.add, ALU.max)
            nc.vector.tensor_mul(nm[:], nm[:], nm[:])
            eq2 = wp.tile([BL, NL], F32, tag="eq2")
            nc.vector.tensor_tensor(
                eq2[:], sm_t[:, Y_L : Y_L + NL], sm_t[:, Y_L : Y_L + NL],
                ALU.is_equal,
            )
            nc.vector.tensor_sub(lab2[:], lab2[:], nm[:])
            nc.vector.tensor_mul(lab2[:], lab2[:], eq2[:])
            nc.vector.tensor_add(lab2[:], lab2[:], nm[:])
            nc.vector.tensor_reduce(
                stk[0:BL, NCHUNK + 1 : NCHUNK + 2], lab2[:], AXIS.X, ALU.add
            )

            # ---------------- recon: sum |x - x_out| -----------------
            # Main chunks: DVE in-place subtract, Act fused Abs+accumulate.
            for i in range(NBIG):
                xt = bp.tile([P, 2, CH], F32, tag="xt")
                nc.sync.dma_start(out=xt[:], in_=xcf1[i])
                nc.vector.tensor_sub(xt[:, 0, :], xt[:, 0, :], xt[:, 1, :])
                nc.scalar.activation(
                    xt[:, 0, :],
                    xt[:, 0, :],
                    AF.Abs,
                    accum_out=stk[:, i : i + 1],
                )
            # Tail chunks: fully on DVE (sub + abs-reduce), no cross-engine
            # hop after the last bytes land.
            for j in range(NSMALL):
                xs = bp.tile([P, 2, CHS], F32, tag="xs")
                nc.sync.dma_start(out=xs[:], in_=xcf2[j])
                nc.vector.tensor_sub(xs[:, 0, :], xs[:, 0, :], xs[:, 1, :])
                nc.vector.tensor_reduce(
                    stk[:, NBIG + j : NBIG + j + 1],
                    xs[:, 0, :],
                    AXIS.X,
                    ALU.add,
                    apply_absolute_value=True,
                )

            nc.sync.dma_start(out=out.ap(), in_=stk[:])

    nc.compile()
    return nc


_NC_CACHE = None


def _get_module():
    global _NC_CACHE
    if _NC_CACHE is None:
        _NC_CACHE = build_module()
    return _NC_CACHE


def make_in_maps(x, x_out, y, mu, disc_pos):
    x = np.ascontiguousarray(x, dtype=np.float32)
    x_out = np.ascontiguousarray(x_out, dtype=np.float32)
    y = np.asarray(y, dtype=np.float32)
    mu = np.asarray(mu, dtype=np.float32)
    disc_pos = np.asarray(disc_pos, dtype=np.float32)

    iota = np.arange(NPOS, dtype=np.float32)
    pos3 = np.tile(np.tile(disc_pos, ND)[None, :], (BL, 1))
    iota3 = np.tile(np.tile(iota, ND)[None, :], (BL, 1))
    iota4 = np.tile(np.tile(iota, NL)[None, :], (BL, 1))

    in_maps = []
    for i in range(N_CORES):
        s = slice(i * BL, (i + 1) * BL)
        xv = x[s].reshape(P, FREE)
        xov = x_out[s].reshape(P, FREE)
        big = NBIG * CH
        xc1 = np.empty((NBIG, P, 2, CH), dtype=np.float32)
        xc1[:, :, 0, :] = xv[:, :big].reshape(P, NBIG, CH).transpose(1, 0, 2)
        xc1[:, :, 1, :] = xov[:, :big].reshape(P, NBIG, CH).transpose(1, 0, 2)
        xc2 = np.empty((NSMALL, P, 2, CHS), dtype=np.float32)
        xc2[:, :, 0, :] = (
            xv[:, big:].reshape(P, NSMALL, CHS).transpose(1, 0, 2)
        )
        xc2[:, :, 1, :] = (
            xov[:, big:].reshape(P, NSMALL, CHS).transpose(1, 0, 2)
        )

        mu_d, mu_l = mu[s, :ND], mu[s, ND : ND + NL]
        y_d, y_l = y[s, :ND], y[s, ND : ND + NL]
        smalls = np.concatenate(
            [
                np.repeat(mu_d, NPOS, axis=1),   # MU_R
                pos3,                            # POS3
                np.repeat(y_d, NPOS, axis=1),    # Y_R
                iota3,                           # IOTA3
                np.repeat(y_l, NPOS, axis=1),    # YL_R
                iota4,                           # IOTA4
                mu_l,                            # MU_L
                y_l,                             # Y_L
                y_d,                             # Y_D
            ],
            axis=1,
        ).astype(np.float32)
        assert smalls.shape == (BL, SM_W)
        in_maps.append(
            {
                "xc1": xc1.reshape(NBIG, P, 2 * CH),
                "xc2": xc2.reshape(NSMALL, P, 2 * CHS),
                "smalls": smalls,
            }
        )
    return in_maps


def combine_partials(partials):
    """partials: [8, P, NCOL] per-core column sums -> full (3,) output."""
    p = np.asarray(partials, dtype=np.float64).reshape(N_CORES, P, NCOL)
    s = p.sum(axis=(0, 1)) / B
    recon = s[:NCHUNK].sum()
    kld = s[NCHUNK] + s[NCHUNK + 1]
    return np.array([recon, kld, recon + kld], dtype=np.float32)


def run_spmd(x, x_out, y, mu, disc_pos, trace=False, **kw):
    from concourse.bass_utils import run_bass_kernel_spmd

    nc = _get_module()
    in_maps = make_in_maps(x, x_out, y, mu, disc_pos)
    r = run_bass_kernel_spmd(nc, in_maps, list(range(N_CORES)), trace=trace, **kw)
    partials = [r.results[i]["out"] for i in range(N_CORES)]
    return combine_partials(partials), r


def kernel(x, x_out, y, mu, disc_pos):
    out, _ = run_spmd(x, x_out, y, mu, disc_pos)
    return out


if __name__ == "__main__":
    nc = build_module()
    print("module built ok")
